# revision 22
# baseline (speedup 1.0000x reference)
"""Chamfer loss kernel for Trainium2 (8 NeuronCores, data-parallel over batch).

Problem: x [32, 2048, 3], y [32, 2048, 3] fp32.
  dist[b, m, n] = ||x[b, n] - y[b, m]||^2
  row[b] = mean_n min_m dist ; col[b] = mean_m min_n dist
  out = mean_b max(row, col)

Per core (4 batches): dist tiles via PE matmul with fp16 hi/lo-split
features (K=13, fp32-accurate): dist = x^2 + y^2 - 2x.y where each
product is split (yh+yl)(uh+ul) ~ yh*uh + yl*uh + yh*ul, u = -2x.

Per m-chunk i (128 rows) a [128, 2048] PSUM group (4 banks) is filled by
4 fp16 matmuls (1 cyc/row at PE). Pool/GPSIMD on this target can only
copy (no ALU ops, no PSUM access), so the drain work is split ACT/DVE:
  ACT copy-casts the PSUM group to an fp16 conv tile (the only non-DVE
  engine that can touch PSUM); DVE computes colmin with a fold chain
  (4 halving 2x-rate fp16 tensor_tensor mins + a [128,128] reduce into
  colsb) and accumulates rowacc with one 2x-rate fp16 tensor_tensor min.
i==0 groups write conv directly into racc (no rowacc op). Deep conv/scr
pools (6/4) keep the ACT->DVE pipeline free of WAR stalls.
Host: rowmin[n] = min_p racc[p, n]; means; max; mean over batch.
"""

import os
import sys

import numpy as np

if "/opt/trn_rl_repo" not in sys.path:
    sys.path.insert(0, "/opt/trn_rl_repo")

B, N, M, D = 32, 2048, 2048, 3
N_CORES = 8
BPC = B // N_CORES  # batches per core = 4
MCH = 16  # m-chunks of 128 per batch
NG = BPC * MCH  # groups per core = 64
KF = 13  # feature rows

# Per-group engine-mix knobs. Pool/GPSIMD on TRN2 can only copy (no ALU
# ops, no PSUM access) and tensor_tensor_reduce does not exist, so DVE is
# the only min-capable engine: conv goes to ACT, colmin to DVE fold chains.
N_DVE_CONV = 0  # conv on DVE (rest on ACT)
N_FOLD = NG  # colmin via DVE fold chain (rest: DVE scan + dma extraction)
EXTRACT = "dma"  # scan last-column extraction: "dma" | "cycle"
SKIP_COLMIN = False  # timing experiment: drop colmin work
SKIP_ROWACC = False  # timing experiment: drop rowacc work
BATCH_REDUCE = False  # batching quartet reduces measured WORSE (130us): cross-group slab deps break pipelining
CONV_BUFS = 6
SCR_BUFS = 4


def _spread(n_special, total=NG):
    picks = set((k * total) // n_special for k in range(n_special)) if n_special else set()
    return [g in picks for g in range(total)]


_CACHE = {}
LAST_RESULTS = None


def _build_bass(repeats=1, hw_loop=False, legalize=True):
    import concourse.bass as bass
    import concourse.tile as tile
    from concourse import mybir

    F32 = mybir.dt.float32
    F16 = mybir.dt.float16
    MIN = mybir.AluOpType.min
    BYP = mybir.AluOpType.bypass
    X = mybir.AxisListType.X
    BIG = 3.0e4

    conv_on_dve = _spread(N_DVE_CONV)
    colmin_fold = _spread(N_FOLD)

    nc = bass.Bass()
    # feats[0] = xfeat [BPC, KF, N], feats[1] = yfeat [BPC, KF, M]
    feats = nc.dram_tensor("feats", [2, BPC, KF, N], F16, kind="ExternalInput")
    # out16[:, : BPC*N] = racc ; out16[:, BPC*N :] = colsb (colmin per group)
    out16 = nc.dram_tensor("out16", [128, BPC * N + NG], F16, kind="ExternalOutput")

    with tile.TileContext(nc) as tc:
        with (
            tc.tile_pool(name="feat", bufs=1) as featp,
            tc.tile_pool(name="psum", bufs=2, space="PSUM") as psump,
            tc.tile_pool(name="racc", bufs=1) as raccp,
            tc.tile_pool(name="conv", bufs=CONV_BUFS) as convp,
            tc.tile_pool(name="scr", bufs=SCR_BUFS) as scrp,
            tc.tile_pool(name="slab", bufs=2) as slabp,
        ):
            ft = featp.tile([KF, 2 * BPC, N], F16, tag="ft")
            in_dmas = []
            for t in range(2):
                for b in range(BPC):
                    d = nc.sync.dma_start(
                        out=ft[:, t * BPC + b, :],
                        in_=feats[t, b],
                    )
                    in_dmas.append(d)
            xft = ft[:, 0:BPC, :]
            yft = ft[:, BPC : 2 * BPC, :]

            racc = raccp.tile([128, BPC * N], F16, tag="racc")
            colsb = raccp.tile([128, NG], F16, tag="colsb")
            dummy = raccp.tile([128, N], F16, tag="dummy")
            nc.vector.memset(dummy[:], 0)
            if SKIP_COLMIN:
                nc.vector.memset(colsb[:], 0)

            extract_cycle = ["dve", "pool", "act"]
            observed = []  # instructions to pre-observe before the drain

            state = {}

            def emit_body(it=0):
                n_ex = 0
                for b in range(BPC):
                    rb = racc[:, b * N : (b + 1) * N]
                    for i in range(MCH):
                        g = b * MCH + i
                        ps = psump.tile([128, N], F32, tag="ps")
                        lhsT = yft[:, b, 128 * i : 128 * (i + 1)]
                        for j in range(4):
                            nc.tensor.matmul(
                                ps[:, 512 * j : 512 * (j + 1)],
                                lhsT,
                                xft[:, b, 512 * j : 512 * (j + 1)],
                                start=True,
                                stop=True,
                            )
                        cv = rb if i == 0 else convp.tile([128, N], F16, tag="cv")
                        gcol = colsb[:, g : g + 1]
                        if conv_on_dve[g]:
                            nc.vector.tensor_copy(cv, ps[:])
                        else:
                            nc.scalar.copy(out=cv, in_=ps[:])
                        scr = scrp.tile([128, N], F16, tag="scr")
                        if SKIP_COLMIN:
                            pass
                        elif not colmin_fold[g]:
                            nc.vector.tensor_tensor_scan(
                                out=scr,
                                data0=cv,
                                data1=dummy[:],
                                initial=BIG,
                                op0=MIN,
                                op1=BYP,
                            )
                            last_col = scr[:, N - 1 : N]
                            if EXTRACT == "dma":
                                d = nc.sync.dma_start(out=gcol, in_=last_col)
                                observed.append(d)
                            else:
                                eng = extract_cycle[n_ex % len(extract_cycle)]
                                n_ex += 1
                                if eng == "dve":
                                    nc.vector.tensor_copy(gcol, last_col)
                                elif eng == "pool":
                                    nc.gpsimd.tensor_copy(gcol, last_col)
                                else:
                                    nc.scalar.copy(out=gcol, in_=last_col)
                        else:  # DVE fold chain on cv
                            nc.vector.tensor_tensor(
                                scr[:, 0:1024], cv[:, 0:1024], cv[:, 1024:2048], MIN
                            )
                            nc.vector.tensor_tensor(
                                scr[:, 1024:1536], scr[:, 0:512],
                                scr[:, 512:1024], MIN,
                            )
                            nc.vector.tensor_tensor(
                                scr[:, 1536:1792], scr[:, 1024:1280],
                                scr[:, 1280:1536], MIN,
                            )
                            if BATCH_REDUCE:
                                if g % 4 == 0:
                                    state["slab"] = slabp.tile(
                                        [128, 4, 128], F16, tag="slab",
                                        name=f"slab_{it}_{g}",
                                    )
                                nc.vector.tensor_tensor(
                                    state["slab"][:, g % 4, :],
                                    scr[:, 1536:1664], scr[:, 1664:1792], MIN,
                                )
                                if g % 4 == 3:
                                    nc.vector.tensor_reduce(
                                        out=colsb[:, g - 3 : g + 1],
                                        in_=state["slab"][:],
                                        axis=X, op=MIN,
                                    )
                            else:
                                nc.vector.tensor_tensor(
                                    scr[:, 1792:1920], scr[:, 1536:1664],
                                    scr[:, 1664:1792], MIN,
                                )
                                nc.vector.tensor_reduce(
                                    out=gcol, in_=scr[:, 1792:1920], axis=X, op=MIN
                                )
                        if i > 0 and not SKIP_ROWACC:
                            nc.vector.tensor_tensor(rb, cv, rb, MIN)
                    # ship this batch's racc in 4 chunks (spread DMA queues)
                    for cchunk in range(4):
                        lo = b * N + 512 * cchunk
                        hi = lo + 512
                        d = nc.sync.dma_start(
                            out=out16[:, lo:hi], in_=racc[:, lo:hi]
                        )
                        observed.append(d)
                dcol = nc.sync.dma_start(
                    out=out16[:, BPC * N :], in_=colsb[:]
                )
                return dcol

            if hw_loop and repeats > 1:
                with tc.For_i(0, repeats, 1):
                    last_dma = emit_body()
            else:
                for r in range(repeats):
                    last_dma = emit_body(r)

            # Pre-observe all DMA lanes except the final colsb DMA so the
            # Tile end-of-kernel Drain needs <=1 sync wait (walrus limit).
            from concourse.tile_rust import add_dep_helper

            for ins in in_dmas + observed:
                nop = nc.sync.nop(nofuse=True)
                add_dep_helper(nop.ins, ins.ins, sync=True, reason="observe dma")

    # Engine instructions support only one sync-wait slot (walrus rejects
    # more). For the end-of-kernel Drain, first drop engine waits (they are
    # redundant with the all-engine barrier that follows). Then hoist excess
    # waits onto same-engine nofuse nops placed just before the instruction;
    # program order carries the dependency.
    for fn in (nc.m.functions if legalize else []):
        for bb in fn.blocks:
            out, changed = [], False
            for ins in bb.instructions:
                si = ins.sync_info
                if si is not None and len(si.on_wait) > 1:
                    waits = list(si.on_wait)
                    if ins.__class__.__name__ == "InstDrain":
                        waits = [x for x in waits if x.ant_name.startswith("DMA")]
                    for k, w in enumerate(waits[:-1]):
                        nop = mybir.InstNoOp(
                            name=f"{ins.name}-lw{k}", engine=ins.engine
                        )
                        nop.sync_info = mybir.SyncInfo(on_wait=[w], on_update=[])
                        nop.bass_nofuse = True
                        nop.bass_priority = ins.bass_priority
                        nop.bass_scheduled_tick = ins.bass_scheduled_tick
                        nop.bass_scheduled_proc = ins.bass_scheduled_proc
                        nop.bass_scheduled_scope = ins.bass_scheduled_scope
                        nop.debug = ins.debug
                        out.append(nop)
                    si.on_wait = waits[-1:]
                    changed = True
                out.append(ins)
            if changed:
                bb.instructions = out

    return nc


def _prep_core_inputs(x, y, c):
    xb = x[BPC * c : BPC * (c + 1)].astype(np.float32)  # [4, N, 3]
    yb = y[BPC * c : BPC * (c + 1)].astype(np.float32)
    u = -2.0 * xb  # [4, N, 3]
    uh = u.astype(np.float16)
    ul = (u - uh.astype(np.float32)).astype(np.float16)
    yh = yb.astype(np.float16)
    yl = (yb - yh.astype(np.float32)).astype(np.float16)
    xsq = np.sum(xb * xb, axis=-1)  # [4, N]
    ysq = np.sum(yb * yb, axis=-1)
    xsqh = xsq.astype(np.float16)
    xsql = (xsq - xsqh.astype(np.float32)).astype(np.float16)
    ysqh = ysq.astype(np.float16)
    ysql = (ysq - ysqh.astype(np.float32)).astype(np.float16)
    ones = np.ones((BPC, N), np.float16)
    zeros = np.zeros((BPC, N), np.float16)
    # sum_k yfeat[k] * xfeat[k] = y^2 + x^2 - 2 x.y (up to yl*ul, ~1e-7)
    xfeat = np.stack(
        [uh[..., 0], uh[..., 1], uh[..., 2],
         uh[..., 0], uh[..., 1], uh[..., 2],
         ul[..., 0], ul[..., 1], ul[..., 2],
         xsqh, xsql, ones, ones], axis=1,
    ).astype(np.float16)  # [4, KF, N]
    yfeat = np.stack(
        [yh[..., 0], yh[..., 1], yh[..., 2],
         yl[..., 0], yl[..., 1], yl[..., 2],
         yh[..., 0], yh[..., 1], yh[..., 2],
         ones, ones, ysqh, ysql], axis=1,
    ).astype(np.float16)
    return np.ascontiguousarray(np.stack([xfeat, yfeat], axis=0))  # [2, 4, KF, N]


def _postprocess(res_list):
    cham = np.zeros((B,), np.float64)
    for c in range(N_CORES):
        out = res_list[c]["out16"]  # [128, BPC*N + NG] fp16
        rowacc = out[:, : BPC * N].reshape(128, BPC, N)
        colsb = out[:, BPC * N :].reshape(128, BPC, MCH)
        rowmin = rowacc.min(axis=0).astype(np.float64)  # [4, N]
        row = rowmin.mean(axis=1)  # [4]
        col = colsb.astype(np.float64).mean(axis=(0, 2))  # [4]
        for b in range(BPC):
            cham[BPC * c + b] = max(row[b], col[b])
    return np.float32(cham.mean())


def kernel(x, y):
    global LAST_RESULTS
    from concourse.bass_utils import run_bass_kernel_spmd

    x = np.asarray(x, dtype=np.float32)
    y = np.asarray(y, dtype=np.float32)
    assert x.shape == (B, N, D) and y.shape == (B, M, D)

    if "nc" not in _CACHE:
        _CACHE["nc"] = _build_bass()
    nc = _CACHE["nc"]

    in_maps = [{"feats": _prep_core_inputs(x, y, c)} for c in range(N_CORES)]
    res = run_bass_kernel_spmd(nc, in_maps, core_ids=list(range(N_CORES)))
    LAST_RESULTS = res
    return _postprocess(res.results)
